# revision 1
# baseline (speedup 1.0000x reference)
"""AttentiveFP forward pass, data-parallel across 8 Trainium2 NeuronCores.

Sharding: batch dim B=256 -> 8 shards of 32 molecules (pure data parallel,
weights replicated). The mol-phase BatchNorm uses full-batch statistics, so
the per-core shards exchange E[ctx] / E[ctx^2] via an 8-core all-reduce
(jax.lax.pmean over the device axis).

Self-contained: hardcodes all shapes from the problem spec; no file reads.
"""

import numpy as np
import jax
import jax.numpy as jnp
from functools import partial

B, L, M = 256, 128, 6
AF, BF, D, R, T, U, O = 39, 10, 128, 3, 2, 64, 1
NEG = -9e8
NCORES = 8

_WEIGHT_NAMES = [
    "atom_fc_W", "atom_fc_b", "neighbor_fc_W", "neighbor_fc_b",
    "gru_Wih", "gru_Whh", "gru_bih", "gru_bhh",
    "align_W", "align_b", "attend_W", "attend_b",
    "mol_gru_Wih", "mol_gru_Whh", "mol_gru_bih", "mol_gru_bhh",
    "mol_align_W", "mol_align_b", "mol_attend_W", "mol_attend_b",
    "bn_gamma", "bn_beta", "mol_output_W", "mol_output_b",
    "output_W", "output_b",
]


def _lrelu(v):
    return jnp.where(v > 0, v, 0.01 * v)


def _elu(v):
    return jnp.where(v > 0, v, jnp.expm1(v))


def _gru(x, h, Wih, Whh, bih, bhh):
    gi = x @ Wih + bih
    gh = h @ Whh + bhh
    ir, iz, i_n = jnp.split(gi, 3, axis=-1)
    hr, hz, h_n = jnp.split(gh, 3, axis=-1)
    r = jax.nn.sigmoid(ir + hr)
    zg = jax.nn.sigmoid(iz + hz)
    ng = jnp.tanh(i_n + r * h_n)
    return (1.0 - zg) * ng + zg * h


def _shard_forward(x_atom, x_bond, x_atom_index, x_bond_index, x_mask, w):
    """Forward for one shard of 32 molecules; BN stats all-reduced over cores."""
    gather = jax.vmap(lambda feat, idx: feat[idx])
    mask3 = x_mask[..., None]
    Lm = x_atom.shape[1]
    atom_feature = _lrelu(x_atom @ w["atom_fc_W"] + w["atom_fc_b"])
    nbr = jnp.concatenate(
        [gather(x_atom, x_atom_index), gather(x_bond, x_bond_index)], axis=-1
    )
    nbr = _lrelu(nbr @ w["neighbor_fc_W"] + w["neighbor_fc_b"])
    attend_mask = (x_atom_index != Lm - 1).astype(x_atom.dtype)[..., None]
    softmax_mask = jnp.where(x_atom_index == Lm - 1, NEG, 0.0).astype(x_atom.dtype)[
        ..., None
    ]

    h = atom_feature
    cur = atom_feature
    for d in range(R):
        expand = jnp.broadcast_to(cur[:, :, None, :], nbr.shape)
        score = (
            _lrelu(
                jnp.concatenate([expand, nbr], -1) @ w["align_W"][d]
                + w["align_b"][d]
            )
            + softmax_mask
        )
        aw = jax.nn.softmax(score, axis=-2) * attend_mask
        ctx = _elu(
            jnp.sum(aw * (nbr @ w["attend_W"][d] + w["attend_b"][d]), axis=-2)
        )
        b_, l_, dd = h.shape
        h = _gru(
            ctx.reshape(-1, dd), h.reshape(-1, dd),
            w["gru_Wih"][d], w["gru_Whh"][d], w["gru_bih"][d], w["gru_bhh"][d],
        ).reshape(b_, l_, dd)
        cur = jax.nn.relu(h)
        if d + 1 < R:
            nbr = gather(cur, x_atom_index)

    mol_feature = jnp.sum(cur * mask3, axis=-2)
    act_mol = jax.nn.relu(mol_feature)
    mol_mask = jnp.where(mask3 == 0, NEG, 0.0)
    for _ in range(T):
        exp_m = jnp.broadcast_to(act_mol[:, None, :], cur.shape)
        sc = (
            _lrelu(
                jnp.concatenate([exp_m, cur], -1) @ w["mol_align_W"]
                + w["mol_align_b"]
            )
            + mol_mask
        )
        aw = jax.nn.softmax(sc, axis=-2) * mask3
        ctx = _elu(
            jnp.sum(aw * (cur @ w["mol_attend_W"] + w["mol_attend_b"]), axis=-2)
        )
        # Full-batch BN statistics: all-reduce the per-shard moments.
        mu = jax.lax.pmean(jnp.mean(ctx, axis=0), axis_name="cores")
        ex2 = jax.lax.pmean(jnp.mean(ctx * ctx, axis=0), axis_name="cores")
        var = ex2 - mu * mu
        ctx = w["bn_gamma"] * (ctx - mu) * jax.lax.rsqrt(var + 1e-5) + w["bn_beta"]
        mol_feature = _gru(
            ctx, mol_feature,
            w["mol_gru_Wih"], w["mol_gru_Whh"], w["mol_gru_bih"], w["mol_gru_bhh"],
        )
        act_mol = jax.nn.relu(mol_feature)

    xo = (mol_feature @ w["mol_output_W"] + w["mol_output_b"]) @ w["output_W"] + w[
        "output_b"
    ]
    return xo, jax.nn.sigmoid(xo)


_pmapped = None


def _get_pmapped():
    global _pmapped
    if _pmapped is None:
        fn = jax.pmap(
            _shard_forward,
            axis_name="cores",
            in_axes=(0, 0, 0, 0, 0, None),
            devices=jax.devices()[:NCORES],
        )
        _pmapped = fn
    return _pmapped


def kernel(**inputs):
    x_atom = np.asarray(inputs["x_atom"], np.float32)
    x_bond = np.asarray(inputs["x_bond"], np.float32)
    x_ai = np.asarray(inputs["x_atom_index"]).astype(np.int32)
    x_bi = np.asarray(inputs["x_bond_index"]).astype(np.int32)
    x_mask = np.asarray(inputs["x_mask"], np.float32)

    w = {k: jnp.asarray(np.asarray(inputs[k], np.float32)) for k in _WEIGHT_NAMES}

    shard = lambda a: a.reshape((NCORES, B // NCORES) + a.shape[1:])
    fn = _get_pmapped()
    xo, sig = fn(shard(x_atom), shard(x_bond), shard(x_ai), shard(x_bi),
                 shard(x_mask), w)
    xo = np.asarray(xo, np.float32).reshape(B, O)
    sig = np.asarray(sig, np.float32).reshape(B, O)
    return xo, sig


if __name__ == "__main__":
    rng = np.random.default_rng(0)
    ins = dict(
        x_atom=rng.standard_normal((B, L, AF), dtype=np.float32),
        x_bond=rng.standard_normal((B, 2 * L, BF), dtype=np.float32),
        x_atom_index=rng.integers(0, L, (B, L, M)).astype(np.int64),
        x_bond_index=rng.integers(0, 2 * L, (B, L, M)).astype(np.int64),
        x_mask=np.ones((B, L), np.float32),
        x_chemical_info=rng.standard_normal((B, 10), dtype=np.float32),
    )
    n = lambda s: (rng.standard_normal(s, dtype=np.float32) * 0.05)
    z = lambda s: np.zeros(s, np.float32)
    ins.update(
        atom_fc_W=n((AF, D)), atom_fc_b=z(D),
        neighbor_fc_W=n((AF + BF, D)), neighbor_fc_b=z(D),
        gru_Wih=n((R, D, 3 * D)), gru_Whh=n((R, D, 3 * D)),
        gru_bih=z((R, 3 * D)), gru_bhh=z((R, 3 * D)),
        align_W=n((R, 2 * D, 1)), align_b=z((R, 1)),
        attend_W=n((R, D, D)), attend_b=z((R, D)),
        mol_gru_Wih=n((D, 3 * D)), mol_gru_Whh=n((D, 3 * D)),
        mol_gru_bih=z(3 * D), mol_gru_bhh=z(3 * D),
        mol_align_W=n((2 * D, 1)), mol_align_b=z(1),
        mol_attend_W=n((D, D)), mol_attend_b=z(D),
        bn_gamma=np.ones(D, np.float32), bn_beta=z(D),
        mol_output_W=n((D, U)), mol_output_b=z(U),
        output_W=n((U, O)), output_b=z(O),
    )
    out = kernel(**ins)
    print("smoke ok:", out[0].shape, out[1].shape, out[0][:3, 0], out[1][:3, 0])


# revision 2
# speedup vs baseline: 5.7551x; 5.7551x over previous
"""AttentiveFP forward pass, data-parallel across 8 Trainium2 NeuronCores.

Sharding: batch dim B=256 -> 8 shards of 32 molecules (pure data parallel,
weights replicated). The mol-phase BatchNorm uses full-batch statistics, so
the per-core shards exchange E[ctx] / E[ctx^2] via an 8-core all-reduce
(jax.lax.pmean over the device axis).

Self-contained: hardcodes all shapes from the problem spec; no file reads.
"""

import numpy as np
import jax
import jax.numpy as jnp
from functools import partial

B, L, M = 256, 128, 6
AF, BF, D, R, T, U, O = 39, 10, 128, 3, 2, 64, 1
NEG = -9e8
NCORES = 8

_WEIGHT_NAMES = [
    "atom_fc_W", "atom_fc_b", "neighbor_fc_W", "neighbor_fc_b",
    "gru_Wih", "gru_Whh", "gru_bih", "gru_bhh",
    "align_W", "align_b", "attend_W", "attend_b",
    "mol_gru_Wih", "mol_gru_Whh", "mol_gru_bih", "mol_gru_bhh",
    "mol_align_W", "mol_align_b", "mol_attend_W", "mol_attend_b",
    "bn_gamma", "bn_beta", "mol_output_W", "mol_output_b",
    "output_W", "output_b",
]


def _lrelu(v):
    return jnp.where(v > 0, v, 0.01 * v)


def _elu(v):
    return jnp.where(v > 0, v, jnp.expm1(v))


def _gru(x, h, Wih, Whh, bih, bhh):
    gi = x @ Wih + bih
    gh = h @ Whh + bhh
    ir, iz, i_n = jnp.split(gi, 3, axis=-1)
    hr, hz, h_n = jnp.split(gh, 3, axis=-1)
    r = jax.nn.sigmoid(ir + hr)
    zg = jax.nn.sigmoid(iz + hz)
    ng = jnp.tanh(i_n + r * h_n)
    return (1.0 - zg) * ng + zg * h


def _shard_forward(x_atom, x_bond, x_atom_index, x_bond_index, x_mask, w):
    """Forward for one shard of 32 molecules; BN stats all-reduced over cores."""
    gather = jax.vmap(lambda feat, idx: feat[idx])
    mask3 = x_mask[..., None]
    Lm = x_atom.shape[1]
    atom_feature = _lrelu(x_atom @ w["atom_fc_W"] + w["atom_fc_b"])
    nbr = jnp.concatenate(
        [gather(x_atom, x_atom_index), gather(x_bond, x_bond_index)], axis=-1
    )
    nbr = _lrelu(nbr @ w["neighbor_fc_W"] + w["neighbor_fc_b"])
    attend_mask = (x_atom_index != Lm - 1).astype(x_atom.dtype)[..., None]
    softmax_mask = jnp.where(x_atom_index == Lm - 1, NEG, 0.0).astype(x_atom.dtype)[
        ..., None
    ]

    h = atom_feature
    cur = atom_feature
    for d in range(R):
        expand = jnp.broadcast_to(cur[:, :, None, :], nbr.shape)
        score = (
            _lrelu(
                jnp.concatenate([expand, nbr], -1) @ w["align_W"][d]
                + w["align_b"][d]
            )
            + softmax_mask
        )
        aw = jax.nn.softmax(score, axis=-2) * attend_mask
        ctx = _elu(
            jnp.sum(aw * (nbr @ w["attend_W"][d] + w["attend_b"][d]), axis=-2)
        )
        b_, l_, dd = h.shape
        h = _gru(
            ctx.reshape(-1, dd), h.reshape(-1, dd),
            w["gru_Wih"][d], w["gru_Whh"][d], w["gru_bih"][d], w["gru_bhh"][d],
        ).reshape(b_, l_, dd)
        cur = jax.nn.relu(h)
        if d + 1 < R:
            nbr = gather(cur, x_atom_index)

    mol_feature = jnp.sum(cur * mask3, axis=-2)
    act_mol = jax.nn.relu(mol_feature)
    mol_mask = jnp.where(mask3 == 0, NEG, 0.0)
    for _ in range(T):
        exp_m = jnp.broadcast_to(act_mol[:, None, :], cur.shape)
        sc = (
            _lrelu(
                jnp.concatenate([exp_m, cur], -1) @ w["mol_align_W"]
                + w["mol_align_b"]
            )
            + mol_mask
        )
        aw = jax.nn.softmax(sc, axis=-2) * mask3
        ctx = _elu(
            jnp.sum(aw * (cur @ w["mol_attend_W"] + w["mol_attend_b"]), axis=-2)
        )
        # Full-batch BN statistics: all-reduce the per-shard moments.
        mu = jax.lax.pmean(jnp.mean(ctx, axis=0), axis_name="cores")
        ex2 = jax.lax.pmean(jnp.mean(ctx * ctx, axis=0), axis_name="cores")
        var = ex2 - mu * mu
        ctx = w["bn_gamma"] * (ctx - mu) * jax.lax.rsqrt(var + 1e-5) + w["bn_beta"]
        mol_feature = _gru(
            ctx, mol_feature,
            w["mol_gru_Wih"], w["mol_gru_Whh"], w["mol_gru_bih"], w["mol_gru_bhh"],
        )
        act_mol = jax.nn.relu(mol_feature)

    xo = (mol_feature @ w["mol_output_W"] + w["mol_output_b"]) @ w["output_W"] + w[
        "output_b"
    ]
    return xo, jax.nn.sigmoid(xo)


_pmapped = None


def _get_pmapped():
    global _pmapped
    if _pmapped is None:
        fn = jax.pmap(
            _shard_forward,
            axis_name="cores",
            in_axes=(0, 0, 0, 0, 0, None),
            devices=jax.devices()[:NCORES],
        )
        _pmapped = fn
    return _pmapped


def _warmup():
    """Trigger XLA compilation at import so calls to kernel() don't pay it."""
    try:
        Bs = B // NCORES
        zf = lambda s: np.zeros(s, np.float32)
        w = {}
        w["atom_fc_W"], w["atom_fc_b"] = zf((AF, D)), zf(D)
        w["neighbor_fc_W"], w["neighbor_fc_b"] = zf((AF + BF, D)), zf(D)
        w["gru_Wih"], w["gru_Whh"] = zf((R, D, 3 * D)), zf((R, D, 3 * D))
        w["gru_bih"], w["gru_bhh"] = zf((R, 3 * D)), zf((R, 3 * D))
        w["align_W"], w["align_b"] = zf((R, 2 * D, 1)), zf((R, 1))
        w["attend_W"], w["attend_b"] = zf((R, D, D)), zf((R, D))
        w["mol_gru_Wih"], w["mol_gru_Whh"] = zf((D, 3 * D)), zf((D, 3 * D))
        w["mol_gru_bih"], w["mol_gru_bhh"] = zf(3 * D), zf(3 * D)
        w["mol_align_W"], w["mol_align_b"] = zf((2 * D, 1)), zf(1)
        w["mol_attend_W"], w["mol_attend_b"] = zf((D, D)), zf(D)
        w["bn_gamma"], w["bn_beta"] = np.ones(D, np.float32), zf(D)
        w["mol_output_W"], w["mol_output_b"] = zf((D, U)), zf(U)
        w["output_W"], w["output_b"] = zf((U, O)), zf(O)
        w = {k: jnp.asarray(v) for k, v in w.items()}
        fn = _get_pmapped()
        out = fn(
            np.zeros((NCORES, Bs, L, AF), np.float32),
            np.zeros((NCORES, Bs, 2 * L, BF), np.float32),
            np.zeros((NCORES, Bs, L, M), np.int32),
            np.zeros((NCORES, Bs, L, M), np.int32),
            np.ones((NCORES, Bs, L), np.float32),
            w,
        )
        jax.block_until_ready(out)
    except Exception:
        global _pmapped
        _pmapped = None


_warmup()


def kernel(**inputs):
    x_atom = np.asarray(inputs["x_atom"], np.float32)
    x_bond = np.asarray(inputs["x_bond"], np.float32)
    x_ai = np.asarray(inputs["x_atom_index"]).astype(np.int32)
    x_bi = np.asarray(inputs["x_bond_index"]).astype(np.int32)
    x_mask = np.asarray(inputs["x_mask"], np.float32)

    w = {k: jnp.asarray(np.asarray(inputs[k], np.float32)) for k in _WEIGHT_NAMES}

    shard = lambda a: a.reshape((NCORES, B // NCORES) + a.shape[1:])
    fn = _get_pmapped()
    xo, sig = fn(shard(x_atom), shard(x_bond), shard(x_ai), shard(x_bi),
                 shard(x_mask), w)
    xo = np.asarray(xo, np.float32).reshape(B, O)
    sig = np.asarray(sig, np.float32).reshape(B, O)
    return xo, sig


if __name__ == "__main__":
    rng = np.random.default_rng(0)
    ins = dict(
        x_atom=rng.standard_normal((B, L, AF), dtype=np.float32),
        x_bond=rng.standard_normal((B, 2 * L, BF), dtype=np.float32),
        x_atom_index=rng.integers(0, L, (B, L, M)).astype(np.int64),
        x_bond_index=rng.integers(0, 2 * L, (B, L, M)).astype(np.int64),
        x_mask=np.ones((B, L), np.float32),
        x_chemical_info=rng.standard_normal((B, 10), dtype=np.float32),
    )
    n = lambda s: (rng.standard_normal(s, dtype=np.float32) * 0.05)
    z = lambda s: np.zeros(s, np.float32)
    ins.update(
        atom_fc_W=n((AF, D)), atom_fc_b=z(D),
        neighbor_fc_W=n((AF + BF, D)), neighbor_fc_b=z(D),
        gru_Wih=n((R, D, 3 * D)), gru_Whh=n((R, D, 3 * D)),
        gru_bih=z((R, 3 * D)), gru_bhh=z((R, 3 * D)),
        align_W=n((R, 2 * D, 1)), align_b=z((R, 1)),
        attend_W=n((R, D, D)), attend_b=z((R, D)),
        mol_gru_Wih=n((D, 3 * D)), mol_gru_Whh=n((D, 3 * D)),
        mol_gru_bih=z(3 * D), mol_gru_bhh=z(3 * D),
        mol_align_W=n((2 * D, 1)), mol_align_b=z(1),
        mol_attend_W=n((D, D)), mol_attend_b=z(D),
        bn_gamma=np.ones(D, np.float32), bn_beta=z(D),
        mol_output_W=n((D, U)), mol_output_b=z(U),
        output_W=n((U, O)), output_b=z(O),
    )
    out = kernel(**ins)
    print("smoke ok:", out[0].shape, out[1].shape, out[0][:3, 0], out[1][:3, 0])
